# revision 5
# baseline (speedup 1.0000x reference)
"""Trainium2 Bass kernel for nn_DifferentiableTriShot.

Strategy
--------
Data-parallel over batch: core k handles batch rows [32k, 32k+32).

Per core (B_loc=32), everything is laid out with partition q = b*4 + c
(c = sequence chunk of 1024) and the within-chunk time index s' on the
free dimension, so all per-timestep logic is elementwise on [128, *]
tiles and the equity compounding is a native tensor_tensor_scan
(cumprod) along the free dim, chunk-stitched with tiny PE shift-matmuls.

The feature dot product (the only real FLOPs) runs on the PE: features
are host-transposed to [128, 65536] with two 64-feature stacks on the
partition axis (K=128); each matmul uses a shifted one-hot weight
matrix so its output lands directly on the correct partition q of the
PSUM accumulator — no cross-partition shuffle needed afterwards.

The per-timestep "scan" of the reference is parallel in disguise:
probs_t depends only on step-t inputs, so equity is a cumulative
product of independently computable factors; position_change needs
only probs_{t-1} (a shift).
"""

import os
import numpy as np

B, S, F = 256, 4096, 64
NCORES = 8
BL = B // NCORES           # 32 batch rows per core
CH = 4                     # sequence chunks per batch row
SP = S // CH               # 1024 timesteps per chunk
NPART = BL * CH            # 128 partitions
NJ = 4                     # quarters of SP processed per pipeline stage
TQ = SP // NJ              # 256
NW = 64                    # qA windows per quarter (one per partition pair)
INITIAL_CAPITAL = 500.0
TCOST = 0.0003

# mm_dtype: "float32r" (1 cyc/row, precision TBD on HW) or "float32"
# (4 cyc/row). flag_mode: "e" compares exp(-z) against transformed
# thresholds; "p" compares the tanh-derived sigmoid against raw ones.
CFG = {"mm_dtype": "float32", "flag_mode": "e"}
LAST_EXEC_NS = [None]


def _build_program(sc, cfg):
    """Build the single-core SPMD Bass program. sc holds the scalar
    hyper-parameters baked in as immediates."""
    from contextlib import ExitStack

    import concourse.bass as bass
    import concourse.bacc as bacc
    import concourse.tile as tile
    from concourse import mybir

    alu = mybir.AluOpType
    act = mybir.ActivationFunctionType
    f32 = mybir.dt.float32
    mm_dt = getattr(mybir.dt, cfg["mm_dtype"])

    lt = float(sc["long_threshold"])
    st = float(sc["short_threshold"])
    bps = float(sc["base_position_size"])
    cs = float(sc["conviction_scalar"])
    vi = float(sc["vix_impact"])
    vct = float(sc["vix_collapse_threshold"])
    vst = float(sc["vix_spike_threshold"])
    # p >= lt  <=>  e := exp(-z) <= (1-lt)/lt   (sigmoid is monotone)
    te_long = float(np.float32(np.float64(1.0 - lt) / np.float64(lt)))
    te_short = float(np.float32(np.float64(1.0 - st) / np.float64(st)))

    nc = bacc.Bacc("TRN2", target_bir_lowering=False, debug=False)

    featT2 = nc.dram_tensor("featT2", [NPART, NJ * NW * TQ], f32, kind="ExternalInput").ap()
    cols03 = nc.dram_tensor("cols03", [4, NPART, SP], f32, kind="ExternalInput").ap()
    gumb = nc.dram_tensor("gumb", [3, NPART, SP], f32, kind="ExternalInput").ap()
    bigab = nc.dram_tensor("bigab", [NPART, 256], f32, kind="ExternalInput").ap()
    shifts = nc.dram_tensor("shifts", [3, NPART, NPART], f32, kind="ExternalInput").ap()
    consts = nc.dram_tensor("consts", [NPART, 8], f32, kind="ExternalInput").ap()

    pred_o = nc.dram_tensor("pred_o", [NPART, SP], f32, kind="ExternalOutput").ap()
    eq_o = nc.dram_tensor("eq_o", [NPART, SP], f32, kind="ExternalOutput").ap()
    pos_o = nc.dram_tensor("pos_o", [3, NPART, SP], f32, kind="ExternalOutput").ap()
    psz_o = nc.dram_tensor("psz_o", [NPART, 1], f32, kind="ExternalOutput").ap()

    with tile.TileContext(nc) as tc, ExitStack() as ctx:
        persist = ctx.enter_context(tc.tile_pool(name="persist", bufs=1))
        stream = ctx.enter_context(tc.tile_pool(name="stream", bufs=2))
        qpool = ctx.enter_context(tc.tile_pool(name="qpool", bufs=2))
        zpool = ctx.enter_context(tc.tile_pool(name="zpool", bufs=2, space="PSUM"))
        tpsum = ctx.enter_context(tc.tile_pool(name="tpsum", bufs=1, space="PSUM"))

        # ---- persistent loads ------------------------------------------------
        big_sb = persist.tile([NPART, 256], f32, name="big_sb")
        nc.sync.dma_start(big_sb, bigab)
        shift_sb = persist.tile([NPART, 3, NPART], f32, name="shift_sb")
        nc.sync.dma_start(shift_sb, shifts.rearrange("k p m -> p k m"))
        cst_sb = persist.tile([NPART, 8], f32, name="cst_sb")
        nc.sync.dma_start(cst_sb, consts)
        c03_sb = persist.tile([NPART, 4, SP], f32, name="c03_sb")
        nc.sync.dma_start(c03_sb, cols03.rearrange("k p s -> p k s"))
        g_sb = persist.tile([NPART, 3, SP], f32, name="g_sb")
        nc.sync.dma_start(g_sb, gumb.rearrange("k p s -> p k s"))

        probs_sb = []
        for k in range(3):
            pk = persist.tile([NPART, SP], f32, name=f"probs{k}", tag=f"probs{k}")
            probs_sb.append(pk)
        predst = persist.tile([NPART, SP], f32, name="predst")
        lst = persist.tile([NPART, SP], f32, name="lst")
        eqst = persist.tile([NPART, SP], f32, name="eqst")
        zq = persist.tile([NPART, TQ], f32, name="zq")
        nc.vector.memset(zq, 0.0)
        sr1c0 = persist.tile([NPART, 1], f32, name="sr1c0")
        f0w = persist.tile([NPART, 1], f32, name="f0w")
        pszlast = persist.tile([NPART, 1], f32, name="pszlast")

        mom = c03_sb[:, 0, :]
        vix = c03_sb[:, 1, :]
        vch = c03_sb[:, 2, :]
        mret = c03_sb[:, 3, :]

        v = nc.vector
        sc_e = nc.scalar

        for j in range(NJ):
            jq = slice(j * TQ, (j + 1) * TQ)
            # ---- stream + matmul: z[q, t] = sum_f w[f] * feat[f, q, t] ------
            z_ps = zpool.tile([NPART, TQ], f32, name="z_ps", tag="z_ps")
            for h in range(2):
                ft = stream.tile([NPART, 32 * TQ], f32, name="ft", tag="ft")
                base = j * NW * TQ + h * 32 * TQ
                nc.sync.dma_start(ft, featT2[:, base : base + 32 * TQ])
                for i in range(32):
                    qa = h * 32 + i
                    nc.tensor.matmul(
                        z_ps,
                        lhsT=big_sb[:, 128 - qa : 256 - qa].bitcast(mm_dt),
                        rhs=ft[:, i * TQ : (i + 1) * TQ].bitcast(mm_dt),
                        start=(qa == 0),
                        stop=(qa == 63),
                    )

            # ---- sigmoid pieces --------------------------------------------
            e_t = qpool.tile([NPART, TQ], f32, name="e_t", tag="e_t")
            sc_e.activation(e_t, z_ps, act.Exp, bias=0.0, scale=-1.0)
            th_t = qpool.tile([NPART, TQ], f32, name="th_t", tag="th_t")
            sc_e.activation(th_t, z_ps, act.Tanh, bias=0.0, scale=0.5)
            v.tensor_scalar(predst[:, jq], th_t, 0.5, 0.5, alu.mult, alu.add)

            # ---- flags ------------------------------------------------------
            mgt = qpool.tile([NPART, TQ], f32, name="mgt", tag="mgt")
            v.tensor_scalar(mgt, mom[:, jq], 0.0, None, alu.is_gt)
            mlt = qpool.tile([NPART, TQ], f32, name="mlt", tag="mlt")
            v.tensor_scalar(mlt, mom[:, jq], 0.0, None, alu.is_lt)
            landp = qpool.tile([NPART, TQ], f32, name="landp", tag="landp")
            sandp = qpool.tile([NPART, TQ], f32, name="sandp", tag="sandp")
            if cfg["flag_mode"] == "e":
                v.scalar_tensor_tensor(landp, e_t, te_long, mgt, alu.is_le, alu.mult)
                v.scalar_tensor_tensor(sandp, e_t, te_short, mlt, alu.is_ge, alu.mult)
            else:
                v.scalar_tensor_tensor(landp, predst[:, jq], lt, mgt, alu.is_ge, alu.mult)
                v.scalar_tensor_tensor(sandp, predst[:, jq], st, mlt, alu.is_le, alu.mult)
            c1 = qpool.tile([NPART, TQ], f32, name="c1", tag="c1")
            v.tensor_scalar(c1, vix[:, jq], 30.0, None, alu.is_lt)
            vc = qpool.tile([NPART, TQ], f32, name="vc", tag="vc")
            v.scalar_tensor_tensor(vc, vch[:, jq], -vct, c1, alu.is_lt, alu.mult)
            c2 = qpool.tile([NPART, TQ], f32, name="c2", tag="c2")
            v.tensor_scalar(c2, vix[:, jq], 20.0, None, alu.is_gt)
            vs = qpool.tile([NPART, TQ], f32, name="vs", tag="vs")
            v.scalar_tensor_tensor(vs, vch[:, jq], vst, c2, alu.is_gt, alu.mult)
            fl_long = qpool.tile([NPART, TQ], f32, name="fl_long", tag="fl_long")
            v.tensor_tensor(fl_long, landp, vc, alu.logical_or)
            fl_short = qpool.tile([NPART, TQ], f32, name="fl_short", tag="fl_short")
            v.tensor_tensor(fl_short, sandp, vs, alu.logical_or)
            nls = qpool.tile([NPART, TQ], f32, name="nls", tag="nls")
            v.tensor_tensor(nls, fl_long, fl_short, alu.logical_or)
            fl_none = qpool.tile([NPART, TQ], f32, name="fl_none", tag="fl_none")
            v.tensor_scalar(fl_none, nls, -1.0, 1.0, alu.mult, alu.add)

            # ---- gumbel softmax --------------------------------------------
            eks = []
            for k, flag in enumerate((fl_long, fl_short, fl_none)):
                lg = qpool.tile([NPART, TQ], f32, name=f"lg{k}", tag="lg")
                v.tensor_scalar(lg, flag, 11.0, -10.0, alu.mult, alu.add)
                la = qpool.tile([NPART, TQ], f32, name=f"la{k}", tag="la")
                v.tensor_tensor(la, lg, g_sb[:, k, jq], alu.add)
                ek = qpool.tile([NPART, TQ], f32, name=f"ek{k}", tag=f"ek{k}")
                sc_e.activation(ek, la, act.Exp, bias=0.0, scale=1.0)
                eks.append(ek)
            den = qpool.tile([NPART, TQ], f32, name="den", tag="den")
            v.tensor_tensor(den, eks[0], eks[1], alu.add)
            v.tensor_tensor(den, den, eks[2], alu.add)
            rden = qpool.tile([NPART, TQ], f32, name="rden", tag="rden")
            v.reciprocal(rden, den)
            for k in range(3):
                v.tensor_tensor(probs_sb[k][:, jq], eks[k], rden, alu.mult)
                nc.sync.dma_start(pos_o[k, :, jq], probs_sb[k][:, jq])

            # ---- position size ---------------------------------------------
            ss = qpool.tile([NPART, TQ], f32, name="ss", tag="ss")
            sc_e.activation(ss, th_t, act.Abs, bias=0.0, scale=1.0)
            psz0 = qpool.tile([NPART, TQ], f32, name="psz0", tag="psz0")
            v.tensor_scalar(psz0, ss, cs, bps, alu.mult, alu.add)
            v.tensor_scalar(psz0, psz0, 1.0, 0.2, alu.min, alu.max)
            vcs = qpool.tile([NPART, TQ], f32, name="vcs", tag="vcs")
            v.tensor_tensor(vcs, vc, vs, alu.logical_or)
            pf = qpool.tile([NPART, TQ], f32, name="pf", tag="pf")
            v.tensor_scalar(pf, vcs, vi, 1.0, alu.mult, alu.add)
            psz = qpool.tile([NPART, TQ], f32, name="psz", tag="psz")
            v.tensor_tensor(psz, psz0, pf, alu.mult)
            if j == NJ - 1:
                v.tensor_copy(pszlast, psz[:, TQ - 1 : TQ])

            # ---- strategy return -------------------------------------------
            d01 = qpool.tile([NPART, TQ], f32, name="d01", tag="d01")
            v.tensor_tensor(d01, probs_sb[0][:, jq], probs_sb[1][:, jq], alu.subtract)
            srt = qpool.tile([NPART, TQ], f32, name="srt", tag="srt")
            v.tensor_tensor(srt, psz, d01, alu.mult)
            v.tensor_tensor(srt, srt, mret[:, jq], alu.mult)
            sr1 = qpool.tile([NPART, TQ], f32, name="sr1", tag="sr1")
            v.tensor_scalar(sr1, srt, 1.0, None, alu.add)
            if j == 0:
                v.tensor_copy(sr1c0, sr1[:, 0:1])

            # ---- position change -------------------------------------------
            pc = qpool.tile([NPART, TQ], f32, name="pc", tag="pc")
            for k in range(3):
                dk = qpool.tile([NPART, TQ], f32, name=f"dk{k}", tag="dk")
                v.tensor_tensor(
                    dk[:, 1:TQ],
                    probs_sb[k][:, j * TQ + 1 : (j + 1) * TQ],
                    probs_sb[k][:, j * TQ : (j + 1) * TQ - 1],
                    alu.subtract,
                )
                if j == 0:
                    # stand-in prev = pos0 = (0,0,1); fixed up via the
                    # tail ratio for chunk rows c>0.
                    if k == 2:
                        v.tensor_scalar(dk[:, 0:1], probs_sb[k][:, 0:1], -1.0, None, alu.add)
                    else:
                        v.tensor_copy(dk[:, 0:1], probs_sb[k][:, 0:1])
                else:
                    v.tensor_tensor(
                        dk[:, 0:1],
                        probs_sb[k][:, j * TQ : j * TQ + 1],
                        probs_sb[k][:, j * TQ - 1 : j * TQ],
                        alu.subtract,
                    )
                ak = qpool.tile([NPART, TQ], f32, name=f"ak{k}", tag="ak")
                sc_e.activation(ak, dk, act.Abs, bias=0.0, scale=1.0)
                if k == 0:
                    v.tensor_copy(pc, ak)
                else:
                    v.tensor_tensor(pc, pc, ak, alu.add)

            # ---- equity factor + scan --------------------------------------
            fac = qpool.tile([NPART, TQ], f32, name="fac", tag="fac")
            v.scalar_tensor_tensor(fac, pc, -TCOST, sr1, alu.mult, alu.add)
            if j == 0:
                v.tensor_copy(f0w, fac[:, 0:1])
                init = 1.0
            else:
                init = lst[:, j * TQ - 1 : j * TQ]
            v.tensor_tensor_scan(lst[:, jq], fac, zq, init, alu.mult, alu.add)

            nc.sync.dma_start(pred_o[:, jq], predst[:, jq])

        # ---- tail: cross-chunk stitching -----------------------------------
        # true prev-probs for s'=0 (probs of partition q-1 at s'=1023, pos0
        # for chunk heads), then ratio = f0_true / f0_standin.
        lc = persist.tile([NPART, 3], f32, name="lc")
        fc = persist.tile([NPART, 3], f32, name="fc")
        for k in range(3):
            v.tensor_copy(lc[:, k : k + 1], probs_sb[k][:, SP - 1 : SP])
            v.tensor_copy(fc[:, k : k + 1], probs_sb[k][:, 0:1])
        ps_prev = tpsum.tile([NPART, 3], f32, name="ps_prev", tag="ps_prev")
        nc.tensor.matmul(ps_prev, lhsT=shift_sb[:, 0, :], rhs=lc, start=True, stop=True)
        prev0 = persist.tile([NPART, 3], f32, name="prev0")
        # consts col 3,4,5 = (0, 0, m1): pos0 fill at chunk heads
        v.tensor_tensor(prev0, ps_prev, cst_sb[:, 3:6], alu.add)
        dd0 = persist.tile([NPART, 3], f32, name="dd0")
        v.tensor_tensor(dd0, fc, prev0, alu.subtract)
        ad0 = persist.tile([NPART, 3], f32, name="ad0")
        sc_e.activation(ad0, dd0, act.Abs, bias=0.0, scale=1.0)
        pc0 = persist.tile([NPART, 1], f32, name="pc0")
        v.tensor_reduce(pc0, ad0, mybir.AxisListType.X, alu.add)
        f0t = persist.tile([NPART, 1], f32, name="f0t")
        v.scalar_tensor_tensor(f0t, pc0, -TCOST, sr1c0, alu.mult, alu.add)
        rf0w = persist.tile([NPART, 1], f32, name="rf0w")
        v.reciprocal(rf0w, f0w)
        ratio = persist.tile([NPART, 1], f32, name="ratio")
        v.tensor_tensor(ratio, f0t, rf0w, alu.mult)
        ttrue = persist.tile([NPART, 1], f32, name="ttrue")
        v.tensor_tensor(ttrue, lst[:, SP - 1 : SP], ratio, alu.mult)

        # chunk-prefix products P[q] = prod_{c'<c} T[c'] via 3 shift matmuls
        accs = []
        for i in range(3):
            ps_t = tpsum.tile([NPART, 1], f32, name=f"ps_t{i}", tag=f"ps_t{i}")
            nc.tensor.matmul(ps_t, lhsT=shift_sb[:, i, :], rhs=ttrue, start=True, stop=True)
            acc = persist.tile([NPART, 1], f32, name=f"acc{i}", tag=f"acc{i}")
            v.tensor_tensor(acc, ps_t, cst_sb[:, i : i + 1], alu.add)
            accs.append(acc)
        pfin = persist.tile([NPART, 1], f32, name="pfin")
        v.tensor_tensor(pfin, accs[0], accs[1], alu.mult)
        v.tensor_tensor(pfin, pfin, accs[2], alu.mult)
        v.tensor_tensor(pfin, pfin, ratio, alu.mult)
        v.tensor_scalar(eqst, lst, pfin, INITIAL_CAPITAL, alu.mult, alu.mult)
        nc.sync.dma_start(eq_o, eqst)
        nc.sync.dma_start(psz_o, pszlast)

    nc.compile()
    return nc


def _host_prep(inputs):
    """Shard + lay out the inputs per core. Pure data movement."""
    feats = np.ascontiguousarray(np.asarray(inputs["features"], dtype=np.float32))
    w = np.asarray(inputs["feature_weights"], dtype=np.float32)
    gumbel = np.asarray(inputs["gumbel_noise"], dtype=np.float32)

    bigab = np.zeros((NPART, 256), dtype=np.float32)
    bigab[0:64, 128] = w
    bigab[64:128, 192] = w

    shifts = np.zeros((3, NPART, NPART), dtype=np.float32)
    for k in range(1, 4):
        for p in range(NPART):
            if p % 4 >= k:
                shifts[k - 1, p - k, p] = 1.0

    consts = np.zeros((NPART, 8), dtype=np.float32)
    pidx = np.arange(NPART)
    consts[:, 0] = (pidx % 4 < 1).astype(np.float32)  # m1
    consts[:, 1] = (pidx % 4 < 2).astype(np.float32)  # m2
    consts[:, 2] = (pidx % 4 < 3).astype(np.float32)  # m3
    consts[:, 5] = (pidx % 4 == 0).astype(np.float32)  # pos0 fill (k=2)

    in_maps = []
    for core in range(NCORES):
        fc = feats[core * BL : (core + 1) * BL]                 # [32, 4096, 64]
        f2 = fc.reshape(NPART * SP, F)                          # row = q*1024+s'
        f4 = f2.reshape(NPART, NJ, TQ, F)                       # [q, j, t, f]
        fa = f4[:64].transpose(3, 1, 0, 2)                      # [f, j, qA, t]
        fb = f4[64:].transpose(3, 1, 0, 2)
        featT2 = np.ascontiguousarray(
            np.concatenate([fa, fb], axis=0)
        ).reshape(NPART, NJ * NW * TQ)

        c03 = np.ascontiguousarray(
            f2.reshape(NPART, SP, F)[:, :, 0:4].transpose(2, 0, 1)
        )                                                        # [4, 128, 1024]

        gc = gumbel[:, core * BL : (core + 1) * BL, :]           # [4096, 32, 3]
        g4 = gc.reshape(CH, SP, BL, 3)                           # [c, s', b, k]
        gco = np.ascontiguousarray(g4.transpose(3, 2, 0, 1)).reshape(3, NPART, SP)

        in_maps.append(
            {
                "featT2": featT2,
                "cols03": c03,
                "gumb": gco,
                "bigab": bigab,
                "shifts": shifts,
                "consts": consts,
            }
        )
    return in_maps


def _assemble(results):
    """Gather per-core outputs into full-shape reference outputs."""
    eq_rows, pos_rows, pred_rows, psz_rows = [], [], [], []
    for r in results:
        pred = r["pred_o"].reshape(BL, CH * SP)
        eq = r["eq_o"].reshape(BL, CH * SP)
        pos = r["pos_o"].reshape(3, BL, CH * SP).transpose(1, 2, 0)
        psz = r["psz_o"].reshape(BL, CH)[:, CH - 1]
        pred_rows.append(pred)
        eq_rows.append(eq)
        pos_rows.append(pos)
        psz_rows.append(psz)
    predictions = np.concatenate(pred_rows, axis=0).astype(np.float32)
    eq_body = np.concatenate(eq_rows, axis=0).astype(np.float32)
    pos_body = np.concatenate(pos_rows, axis=0).astype(np.float32)
    position_sizes = np.concatenate(psz_rows, axis=0).astype(np.float32)

    equity = np.empty((B, S + 1), dtype=np.float32)
    equity[:, 0] = INITIAL_CAPITAL
    equity[:, 1:] = eq_body
    positions = np.empty((B, S + 1, 3), dtype=np.float32)
    positions[:, 0, :] = np.array([0.0, 0.0, 1.0], dtype=np.float32)
    positions[:, 1:, :] = pos_body
    return equity, positions, predictions, position_sizes


def kernel(**inputs):
    from concourse import bass_utils

    sc = {
        k: float(np.asarray(inputs[k]))
        for k in (
            "long_threshold",
            "short_threshold",
            "base_position_size",
            "conviction_scalar",
            "vix_impact",
            "vix_collapse_threshold",
            "vix_spike_threshold",
        )
    }
    nc = _build_program(sc, CFG)
    in_maps = _host_prep(inputs)
    trace = bool(int(os.environ.get("TRISHOT_TRACE", "0")))
    res = bass_utils.run_bass_kernel_spmd(
        nc, in_maps, core_ids=list(range(NCORES)), trace=trace
    )
    LAST_EXEC_NS[0] = res.exec_time_ns
    return _assemble(res.results)
